# revision 1
# baseline (speedup 1.0000x reference)
"""Trainium2 kernel for the sobel-perception CNN cell.

Computation (per pixel, circular 3x3 stencil):
    perc = [sobel_x * x, sobel_y * x, x]            # 48 channels
    hidden = relu(W1 @ perc + b1)                   # 128 channels
    out    = W2 @ hidden + b2                       # 16 channels

Key transform: the depthwise sobel convs use the same 2d kernel for all
channels, so they commute with the 1x1 channel-mixing conv.  Folding them
into W1 gives  hidden = relu(sum_{dy,dx} M[dy,dx] @ x_shift(dy,dx) + b1)
with M[dy,dx] = SX[dy,dx]*W1a + SY[dy,dx]*W1b + (dy==0,dx==0)*W1c.
Grouping by dx yields 3 accumulating matmuls of K=48 against a 3-row
stacked copy of x (dy folded into the stack, dx a free-dim offset).

Sharding: rows of the 1024x1024 grid split across 8 cores (128 rows each);
the host bakes the 1-row circular halo and the column wrap into each
core's input slab, so the device kernel needs no collectives.
"""

import sys

sys.path.insert(0, "/opt/trn_rl_repo")

import numpy as np

import concourse.bass as bass
import concourse.mybir as mybir
from concourse.bass_utils import run_bass_kernel_spmd
from concourse.tile import TileContext

H, W, C, HID = 1024, 1024, 16, 128
NCORES = 8
RPC = H // NCORES  # rows per core
CH = 512  # matmul free-dim chunk (one PSUM bank of fp32)
ROWS_PER_GROUP = 2  # rows per supergroup -> 4 chunks -> 4 PE col groups

_SOBEL_X = np.array([[-1.0, 0.0, 1.0], [-2.0, 0.0, 2.0], [-1.0, 0.0, 1.0]], np.float32)
_SOBEL_Y = np.array([[-1.0, -2.0, -1.0], [0.0, 0.0, 0.0], [1.0, 2.0, 1.0]], np.float32)

F32 = mybir.dt.float32
F32R = mybir.dt.float32r


def build_a_mats(W1: np.ndarray) -> np.ndarray:
    """A[dx][o, dy*16+ch] for dx in (-1, 0, +1) -> shape (3, 128, 48)."""
    W1a, W1b, W1c = W1[:, 0:C], W1[:, C : 2 * C], W1[:, 2 * C : 3 * C]
    A = np.zeros((3, HID, 3 * C), np.float32)
    for dxi in range(3):
        for dyi in range(3):
            m = _SOBEL_X[dyi, dxi] * W1a + _SOBEL_Y[dyi, dxi] * W1b
            if dyi == 1 and dxi == 1:
                m = m + W1c
            A[dxi, :, dyi * C : (dyi + 1) * C] = m
    return A


def build_window_mats(W1: np.ndarray):
    """Fold matrices for the 4-row x 2-dx window layout.

    Window partition p = dxv*64 + dr*16 + ch holds xtp[ch, rbase+dr, c+dxv].
    Output row i (i in {0,1} within the pair) uses dy = dr-1-i and
    dx = dxv-1 (mm_a) plus dx=+1 from the dxv=0 block at col offset +2
    (mm_b).  Returns (aa, ab): aa[i] is [128, 128] lhsT for mm_a,
    ab[i] is [64, 128] lhsT for mm_b."""
    A = build_a_mats(W1)  # A[dxi] = M[:, dyi-blocks] as (3, 128, 48)
    aa = np.zeros((2, 128, HID), np.float32)
    ab = np.zeros((2, 64, HID), np.float32)
    for i in range(2):
        for dxv in range(2):  # dx = dxv - 1
            for dr in range(4):
                dyi = dr - i  # dyi = dy+1 in 0..2
                if 0 <= dyi <= 2:
                    blk = A[dxv][:, dyi * C : (dyi + 1) * C]  # (128, 16)
                    aa[i, dxv * 64 + dr * 16 : dxv * 64 + dr * 16 + C] = blk.T
        for dr in range(4):
            dyi = dr - i
            if 0 <= dyi <= 2:
                blk = A[2][:, dyi * C : (dyi + 1) * C]
                ab[i, dr * 16 : dr * 16 + C] = blk.T
    return aa, ab


def _hoist_matmul_waits(nc: bass.Bass) -> None:
    """This walrus build's instruction formats hold at most ONE sync wait,
    but Tile emits 2-3 on some instructions.  Hoist excess waits onto
    inserted same-engine NoOps (one wait each) right before the
    instruction — semantically the same blocking point on the in-order
    engine queue."""
    fixn = 0
    for fn in nc.m.functions:
        for blk in fn.blocks:
            needs_fix = any(
                inst.sync_info is not None and len(inst.sync_info.on_wait) > 1
                for inst in blk.instructions
            )
            if not needs_fix:
                continue
            out = []
            for inst in blk.instructions:
                si = inst.sync_info
                if si is not None and len(si.on_wait) > 1:
                    for w in si.on_wait:
                        nop = mybir.InstNoOp(name=f"I-mmfix-{fixn}")
                        fixn += 1
                        nop.engine = inst.engine
                        nop.sync_info = mybir.SyncInfo(on_wait=[w], on_update=[])
                        out.append(nop)
                    si.on_wait = []
                out.append(inst)
            blk.instructions = out


def build_nc(rpc: int = RPC, w: int = W, ch: int = CH, hoist: bool = True) -> bass.Bass:
    wp = w + 2
    halves = w // ch  # column chunks per row
    assert ROWS_PER_GROUP * halves == 4 and rpc % ROWS_PER_GROUP == 0

    nc = bass.Bass()
    xw = nc.declare_dram_parameter("xw", [128, rpc // 2, wp], F32R, isOutput=False)
    ata = nc.declare_dram_parameter("ata", [128, 2 * HID], F32R, isOutput=False)
    atb = nc.declare_dram_parameter("atb", [64, 2 * HID], F32R, isOutput=False)
    w2t = nc.declare_dram_parameter("w2t", [HID, C], F32R, isOutput=False)
    b1 = nc.declare_dram_parameter("b1", [HID, 1], F32, isOutput=False)
    out = nc.declare_dram_parameter("out", [C, rpc, w], F32, isOutput=True)

    with TileContext(nc) as tc:
        with (
            tc.tile_pool(name="const", bufs=1) as cpool,
            tc.tile_pool(name="xrows", bufs=14) as xpool,
            tc.tile_pool(name="hid", bufs=10) as hpool,
            tc.tile_pool(name="stage", bufs=4) as spool,
            tc.tile_pool(name="cps", bufs=4, space="PSUM") as cps,
            tc.tile_pool(name="ops", bufs=3, space="PSUM") as ops,
        ):
            ata_t = cpool.tile([128, 2 * HID], F32R)
            nc.sync.dma_start(out=ata_t[:], in_=ata[:])
            atb_t = cpool.tile([64, 2 * HID], F32R)
            nc.sync.dma_start(out=atb_t[:], in_=atb[:])
            w2t_t = cpool.tile([HID, C], F32R)
            nc.sync.dma_start(out=w2t_t[:], in_=w2t[:])
            b1_t = cpool.tile([HID, 1], F32)
            nc.sync.dma_start(out=b1_t[:], in_=b1[:])

            for g in range(rpc // ROWS_PER_GROUP):
                win = xpool.tile([128, wp], F32R, tag="xrow", name=f"xw{g}")
                nc.sync.dma_start(out=win[:], in_=xw[:, g, :])

                # conv1: 3 accumulating K=48 matmuls per chunk
                convs = [
                    cps.tile([HID, ch], F32, tag="cv", name=f"cv{g}_{i}")
                    for i in range(4)
                ]
                # chunk c -> output row i = c//halves, column half c%halves.
                # Group by lhsT so the PE reloads weights only 4x per window.
                for i in range(2):
                    for hh in range(halves):
                        c = i * halves + hh
                        base = ch * hh
                        nc.tensor.matmul(
                            convs[c][:],
                            ata_t[:, i * HID : (i + 1) * HID],
                            win[:, base : base + ch],
                            start=True, stop=False,
                        )
                for i in range(2):
                    for hh in range(halves):
                        c = i * halves + hh
                        base = ch * hh
                        nc.tensor.matmul(
                            convs[c][:],
                            atb_t[:, i * HID : (i + 1) * HID],
                            win[0:64, base + 2 : base + 2 + ch],
                            start=False, stop=True,
                        )

                # bias + relu, PSUM -> SBUF
                hids = []
                for c in range(4):
                    ht = hpool.tile([HID, ch], F32R, tag="h", name=f"h{g}_{c}")
                    nc.scalar.activation(
                        ht[:],
                        convs[c][:],
                        mybir.ActivationFunctionType.Relu,
                        bias=b1_t[:],
                        scale=1.0,
                    )
                    hids.append(ht)

                # mm2: K=128 -> M=16 per chunk
                st = spool.tile([HID, ch], F32, tag="st", name=f"st{g}")
                for c in range(4):
                    ot = ops.tile([C, ch], F32, tag="o", name=f"o{g}_{c}")
                    nc.tensor.matmul(
                        ot[:],
                        w2t_t[:],
                        hids[c][:],
                        start=True,
                        stop=True,
                    )
                    nc.vector.tensor_copy(st[32 * c : 32 * c + C, :], ot[:])

                for c in range(4):
                    r = g * ROWS_PER_GROUP + c // halves
                    col0 = ch * (c % halves)
                    nc.gpsimd.dma_start(
                        out=out[:, r, col0 : col0 + ch],
                        in_=st[32 * c : 32 * c + C, :],
                    )

    if hoist:
        _hoist_matmul_waits(nc)
    return nc


_NC_CACHE: dict = {}


def _get_nc():
    if "nc" not in _NC_CACHE:
        _NC_CACHE["nc"] = build_nc()
    return _NC_CACHE["nc"]


def host_prepare(state, W1, b1, W2):
    """Build per-core input maps. state: (H, W, C) f32."""
    xt = np.ascontiguousarray(state.transpose(2, 0, 1))  # (C, H, W)
    xtp = np.pad(xt, ((0, 0), (1, 1), (1, 2)), mode="wrap")  # (C, H+2, W+3)
    aa, ab = build_window_mats(W1)
    ata = np.ascontiguousarray(np.concatenate([aa[0], aa[1]], axis=1))  # (128, 256)
    atb = np.ascontiguousarray(np.concatenate([ab[0], ab[1]], axis=1))  # (64, 256)
    w2t = np.ascontiguousarray(W2.T)  # (128, 16)
    b1c = np.ascontiguousarray(b1.reshape(HID, 1))

    in_maps = []
    for k in range(NCORES):
        r0 = k * RPC
        # xw[dxv*64+dr*16+ch, w, cp] = xtp[ch, r0+2w+dr, cp+dxv]
        nw = RPC // 2
        s = np.empty((128, nw, W + 2), np.float32)
        for dxv in range(2):
            for dr in range(4):
                p0 = dxv * 64 + dr * 16
                s[p0 : p0 + C] = xtp[
                    :, r0 + dr : r0 + dr + 2 * nw : 2, dxv : dxv + W + 2
                ]
        in_maps.append(
            {
                "xw": np.ascontiguousarray(s),
                "ata": ata,
                "atb": atb,
                "w2t": w2t,
                "b1": b1c,
            }
        )
    return in_maps


def kernel(state, W1, b1, W2, b2, **extra):
    state = np.asarray(state, np.float32)
    W1 = np.asarray(W1, np.float32)
    b1 = np.asarray(b1, np.float32)
    W2 = np.asarray(W2, np.float32)
    b2 = np.asarray(b2, np.float32)

    nc = _get_nc()
    in_maps = host_prepare(state, W1, b1, W2)
    res = run_bass_kernel_spmd(nc, in_maps, core_ids=list(range(NCORES)))

    out_t = np.concatenate([res.results[k]["out"] for k in range(NCORES)], axis=1)
    out = out_t.transpose(1, 2, 0) + b2[None, None, :]
    return np.ascontiguousarray(out.astype(np.float32))


if __name__ == "__main__":
    rng = np.random.default_rng(0)
    state = rng.standard_normal((H, W, C), dtype=np.float32)
    W1 = rng.standard_normal((HID, 3 * C), dtype=np.float32) * 0.1
    b1v = rng.standard_normal(HID).astype(np.float32) * 0.1
    W2 = rng.standard_normal((C, HID), dtype=np.float32) * 0.1
    b2v = rng.standard_normal(C).astype(np.float32) * 0.1
    out = kernel(state, W1, b1v, W2, b2v)
    print(out.shape, out.dtype)



# revision 4
# speedup vs baseline: 1.2650x; 1.2650x over previous
"""Trainium2 kernel for the sobel-perception CNN cell.

Computation (per pixel, circular 3x3 stencil):
    perc = [sobel_x * x, sobel_y * x, x]            # 48 channels
    hidden = relu(W1 @ perc + b1)                   # 128 channels
    out    = W2 @ hidden + b2                       # 16 channels

The depthwise sobel convs share one 2d kernel across channels, so they
commute with the 1x1 channel-mixing conv: folding them into W1 gives
hidden = relu(sum_{dy,dx} M[dy,dx] @ x_shift(dy,dx) + b1).

Device layout (v2):
  * 4-row window per output-row-pair: partitions hold [dr(4) x ch(16)] = 64,
    dy folded into the partition stack, dx a free-dim column offset.
    conv1 = 3 accumulating K=64 matmuls (dx = -1, 0, +1).
  * TWO windows stacked at partition bases 0 / 64 -> PE row-tiling runs the
    two K=64 matmul chains concurrently (2x effective matmul rate).
  * mm2 (M=16) packs 4 chunks into one PSUM bank via 4x column tiling
    (tile_position cols 0/32/64/96) -> 4 chunks copied out per [128,512] op.
  * bf16 operands: FWL weight loads, half the HBM traffic; PSUM stays f32.
  * relu+bias PSUM->SBUF split between Scalar (activation) and Vector
    (tensor_scalar add-bias/max-0) engines.

Sharding: rows of the 1024x1024 grid split across 8 cores (128 rows each);
the host bakes the circular halos into each core's window slab (2x row
replication), so the device kernel needs no collectives.
"""

import sys

sys.path.insert(0, "/opt/trn_rl_repo")

import ml_dtypes
import numpy as np

import concourse.bass as bass
import concourse.mybir as mybir
from concourse.bass_utils import run_bass_kernel_spmd
from concourse.tile import TileContext

H, W, C, HID = 1024, 1024, 16, 128
NCORES = 8
RPC = H // NCORES  # rows per core
SG = RPC // 4  # supergroups per core (4 output rows each)
WP = W + 2  # padded columns
CH = 512  # matmul free-dim chunk (one PSUM bank of fp32)

_SOBEL_X = np.array([[-1.0, 0.0, 1.0], [-2.0, 0.0, 2.0], [-1.0, 0.0, 1.0]], np.float32)
_SOBEL_Y = np.array([[-1.0, -2.0, -1.0], [0.0, 0.0, 0.0], [1.0, 2.0, 1.0]], np.float32)

F32 = mybir.dt.float32
BF16 = mybir.dt.bfloat16
NPBF16 = ml_dtypes.bfloat16


def build_a_mats(W1: np.ndarray) -> np.ndarray:
    """A[dx][o, dy*16+ch] for dx in (-1, 0, +1) -> shape (3, 128, 48)."""
    W1a, W1b, W1c = W1[:, 0:C], W1[:, C : 2 * C], W1[:, 2 * C : 3 * C]
    A = np.zeros((3, HID, 3 * C), np.float32)
    for dxi in range(3):
        for dyi in range(3):
            m = _SOBEL_X[dyi, dxi] * W1a + _SOBEL_Y[dyi, dxi] * W1b
            if dyi == 1 and dxi == 1:
                m = m + W1c
            A[dxi, :, dyi * C : (dyi + 1) * C] = m
    return A


def build_wt(W1: np.ndarray) -> np.ndarray:
    """lhsT slab [128, 6*128]: wt[64t + dr*16+ch, (dxi*2+i)*128 + o] =
    M[dy=dr-1-i, dx=dxi-1][o, ch] (zero outside 0<=dr-i<=2); both 64-row
    halves (t=0,1) hold the same content for the two PE row-tiles."""
    A = build_a_mats(W1)
    wt = np.zeros((128, 6 * HID), np.float32)
    for dxi in range(3):
        for i in range(2):
            col0 = (dxi * 2 + i) * HID
            for dr in range(4):
                dyi = dr - i
                if 0 <= dyi <= 2:
                    blk = A[dxi][:, dyi * C : (dyi + 1) * C]  # (128, 16)
                    for t in range(2):
                        p0 = 64 * t + dr * C
                        wt[p0 : p0 + C, col0 : col0 + HID] = blk.T
    return wt


def _hoist_matmul_waits(nc: bass.Bass) -> None:
    """This walrus build's instruction formats hold at most ONE sync wait,
    but Tile emits 2-3 on some instructions.  Hoist excess waits onto
    inserted same-engine NoOps (one wait each) right before the
    instruction — semantically the same blocking point on the in-order
    engine queue."""
    fixn = 0
    for fn in nc.m.functions:
        for blk in fn.blocks:
            needs_fix = any(
                inst.sync_info is not None and len(inst.sync_info.on_wait) > 1
                for inst in blk.instructions
            )
            if not needs_fix:
                continue
            out = []
            for inst in blk.instructions:
                si = inst.sync_info
                if si is not None and len(si.on_wait) > 1:
                    for w in si.on_wait:
                        nop = mybir.InstNoOp(name=f"I-mmfix-{fixn}")
                        fixn += 1
                        nop.engine = inst.engine
                        nop.sync_info = mybir.SyncInfo(on_wait=[w], on_update=[])
                        out.append(nop)
                    si.on_wait = []
                out.append(inst)
            blk.instructions = out


def build_nc(hoist: bool = True) -> bass.Bass:
    nc = bass.Bass()
    xw = nc.declare_dram_parameter("xw", [128, SG, WP], BF16, isOutput=False)
    wt = nc.declare_dram_parameter("wt", [128, 6 * HID], BF16, isOutput=False)
    w2t = nc.declare_dram_parameter("w2t", [HID, C], BF16, isOutput=False)
    b1 = nc.declare_dram_parameter("b1", [HID, 1], F32, isOutput=False)
    out = nc.declare_dram_parameter("out", [C, RPC, W], F32, isOutput=True)

    with TileContext(nc) as tc:
        with (
            tc.tile_pool(name="const", bufs=1) as cpool,
            tc.tile_pool(name="xrows", bufs=6) as xpool,
            tc.tile_pool(name="hid", bufs=8) as hpool,
            tc.tile_pool(name="stage", bufs=6) as spool,
            tc.tile_pool(name="cps", bufs=3, space="PSUM") as cps,
            tc.tile_pool(name="ops", bufs=2, space="PSUM") as ops,
        ):
            wt_t = cpool.tile([128, 6 * HID], BF16)
            nc.sync.dma_start(out=wt_t[:], in_=wt[:])
            w2t_t = cpool.tile([HID, C], BF16)
            nc.sync.dma_start(out=w2t_t[:], in_=w2t[:])
            b1_t = cpool.tile([HID, 1], F32)
            nc.sync.dma_start(out=b1_t[:], in_=b1[:])

            for s in range(SG):
                win = xpool.tile([128, WP], BF16, tag="xrow", name=f"xw{s}")
                nc.sync.dma_start(out=win[:], in_=xw[:, s, :])

                # conv1: per (t, i): [128, 1024] PSUM tile, 3 accumulating
                # K=64 matmuls per 512-chunk; t=0/1 row-tiles run concurrently
                cv = {}
                for i in range(2):
                    for t in range(2):
                        cv[(t, i)] = cps.tile(
                            [128, 2 * CH], F32, tag="cv", name=f"cv{s}_{t}{i}"
                        )
                for i in range(2):
                    for h in range(2):
                        for dxi in range(3):
                            for t in range(2):
                                nc.tensor.matmul(
                                    cv[(t, i)][:, h * CH : (h + 1) * CH],
                                    wt_t[
                                        64 * t : 64 * t + 64,
                                        (dxi * 2 + i) * HID : (dxi * 2 + i + 1) * HID,
                                    ],
                                    win[64 * t : 64 * t + 64, h * CH + dxi : h * CH + dxi + CH],
                                    start=(dxi == 0),
                                    stop=(dxi == 2),
                                    tile_position=(64 * t, 0),
                                )

                # bias + relu, PSUM -> SBUF bf16; split scalar/vector engines
                hid = {}
                for t in range(2):
                    for i in range(2):
                        ht = hpool.tile([128, 2 * CH], BF16, tag="h", name=f"h{s}_{t}{i}")
                        if i == 0:
                            nc.scalar.activation(
                                ht[:],
                                cv[(t, i)][:],
                                mybir.ActivationFunctionType.Relu,
                                bias=b1_t[:],
                                scale=1.0,
                            )
                        else:
                            nc.vector.tensor_scalar(
                                ht[:],
                                cv[(t, i)][:],
                                b1_t[:],
                                0.0,
                                mybir.AluOpType.add,
                                mybir.AluOpType.max,
                            )
                        hid[(t, i)] = ht

                # mm2: per t, 4 col-tiled chunks into one PSUM bank
                for t in range(2):
                    ot = ops.tile([128, CH], F32, tag="o", name=f"o{s}_{t}")
                    for i in range(2):
                        for h in range(2):
                            j = 2 * i + h
                            nc.tensor.matmul(
                                ot[32 * j : 32 * j + C, :],
                                w2t_t[:],
                                hid[(t, i)][:, h * CH : (h + 1) * CH],
                                start=True,
                                stop=True,
                                tile_position=(0, 32 * j),
                            )
                    st = spool.tile([128, CH], F32, tag="st", name=f"st{s}_{t}")
                    if t == 0:
                        nc.scalar.activation(
                            st[:], ot[:], mybir.ActivationFunctionType.Copy,
                            bias=0.0, scale=1.0,
                        )
                    else:
                        nc.vector.tensor_copy(st[:], ot[:])
                    for i in range(2):
                        for h in range(2):
                            j = 2 * i + h
                            r = 4 * s + 2 * t + i
                            nc.gpsimd.dma_start(
                                out=out[:, r, h * CH : (h + 1) * CH],
                                in_=st[32 * j : 32 * j + C, :],
                            )

    if hoist:
        _hoist_matmul_waits(nc)
    return nc


_NC_CACHE: dict = {}


def _get_nc():
    if "nc" not in _NC_CACHE:
        _NC_CACHE["nc"] = build_nc()
    return _NC_CACHE["nc"]


def host_prepare(state, W1, b1, W2):
    """Build per-core input maps. state: (H, W, C) f32."""
    xt = np.ascontiguousarray(state.transpose(2, 0, 1))  # (C, H, W)
    xtp = np.pad(xt, ((0, 0), (1, 1), (1, 1)), mode="wrap")  # (C, H+2, W+2)
    xtp_bf = xtp.astype(NPBF16)
    wt = build_wt(W1).astype(NPBF16)
    w2t = np.ascontiguousarray(W2.T).astype(NPBF16)  # (128, 16)
    b1c = np.ascontiguousarray(b1.reshape(HID, 1)).astype(np.float32)

    in_maps = []
    for k in range(NCORES):
        r0 = k * RPC
        # xw[64t + dr*16 + ch, s, c] = xtp[ch, r0 + 4s + 2t + dr, c]
        slab = np.empty((128, SG, WP), NPBF16)
        for t in range(2):
            for dr in range(4):
                p0 = 64 * t + dr * C
                rbase = r0 + 2 * t + dr
                slab[p0 : p0 + C] = xtp_bf[:, rbase : rbase + 4 * SG : 4, :]
        in_maps.append(
            {
                "xw": np.ascontiguousarray(slab),
                "wt": wt,
                "w2t": w2t,
                "b1": b1c,
            }
        )
    return in_maps


def kernel(state, W1, b1, W2, b2, **extra):
    state = np.asarray(state, np.float32)
    W1 = np.asarray(W1, np.float32)
    b1 = np.asarray(b1, np.float32)
    W2 = np.asarray(W2, np.float32)
    b2 = np.asarray(b2, np.float32)

    nc = _get_nc()
    in_maps = host_prepare(state, W1, b1, W2)
    res = run_bass_kernel_spmd(nc, in_maps, core_ids=list(range(NCORES)))

    out_t = np.concatenate([res.results[k]["out"] for k in range(NCORES)], axis=1)
    out = out_t.transpose(1, 2, 0) + b2[None, None, :]
    return np.ascontiguousarray(out.astype(np.float32))


if __name__ == "__main__":
    rng = np.random.default_rng(0)
    state = rng.standard_normal((H, W, C), dtype=np.float32)
    W1 = rng.standard_normal((HID, 3 * C), dtype=np.float32) * 0.1
    b1v = rng.standard_normal(HID).astype(np.float32) * 0.1
    W2 = rng.standard_normal((C, HID), dtype=np.float32) * 0.1
    b2v = rng.standard_normal(C).astype(np.float32) * 0.1
    out = kernel(state, W1, b1v, W2, b2v)
    print(out.shape, out.dtype)
